# revision 13
# baseline (speedup 1.0000x reference)
"""Trainium2 Bass kernel for nn_MaxAssigner2D (span=2 shifted channel-max pool).

Math (per image, zero-padded borders):
    m[h, w]   = max_c x[h, w, c]
    out[h, w] = max over (dh, dw) in S of m[h-dh, w-dw]   (0 outside bounds)
    S = {(0,0), (1,0), (0,1), (1,1), (2,0), (0,2), (2,2)}

Distribution: pure data parallel, 2 images per core across 8 NeuronCores.

Per-core layout: partition p <-> 4-row band of the image (128 bands x 4 rows
= 512 rows).  The channel-max plane m for each image lives in SBUF as
[128, 6*514] f32: per partition 2 halo rows (rows 4p-2, 4p-1, zero for p=0)
followed by the band's 4 rows, each row stored as [2 zero pad cols | 512 data
cols].  With that layout every shift (dh, dw) of the 7-term max is a pure
free-dim AP offset, and the zero padding of the reference comes for free.

Pipeline per 128-row x-chunk (16 KB/partition contiguous DMA):
    nc.sync DMA in -> DVE tensor_max level-1 (32ch -> 16ch, eats 2 elem/cyc)
    -> GPSIMD tensor_max tree (16 -> 1) writing into the m tile.
Then per image: partition-shifted SBUF->SBUF DMA fills halo rows, 6 DVE
tensor_max ops accumulate the 7 shifts, DMA out.
"""

import numpy as np

import concourse.bacc as bacc
import concourse.bass as bass
import concourse.mybir as mybir
from concourse.tile import TileContext

F32 = mybir.dt.float32
NCORES = 8

# Full-problem geometry (hardcoded; kernel.py must be self-contained).
B, H, W, C = 16, 512, 512, 32
SPAN = 2


def build_nc(bpc, h, w, c, ph, qw):
    """Build the per-core Bass module.

    bpc: images per core; h/w/c: image dims; ph: rows per band (partitions =
    h // ph); qw: pixels per stage-1 chunk per partition.
    """
    P = h // ph               # partitions used
    assert P <= 128
    nq = w // qw              # chunks per band row
    rowp = SPAN + w           # padded row width (left zero pad only)
    mrows = ph + SPAN         # halo rows + band rows
    msz = mrows * rowp

    # Bacc (not raw Bass): its finalize() runs generate_event_semaphores(),
    # which splits multi-wait instructions to satisfy the TRN2 1-wait limit.
    nc = bacc.Bacc("TRN2")
    x = nc.declare_dram_parameter("x", [bpc, h, w, c], F32, isOutput=False)
    out = nc.declare_dram_parameter("out", [bpc, h, w, 1], F32, isOutput=True)

    # DRAM views: partition p <-> band p
    xr = x.ap().rearrange("b (p ph) w c -> b p ph (w c)", ph=ph)     # [bpc,P,ph,w*c]
    outr = out.ap().rearrange("b (p ph) w c -> b p (ph w c)", ph=ph)  # [bpc,P,ph*w]

    with TileContext(nc) as tc:
        with (
            tc.tile_pool(name="xp", bufs=3) as xpool,
            tc.tile_pool(name="mp", bufs=1) as mpool,
            tc.tile_pool(name="op", bufs=2) as opool,
        ):
            # Persistent per-image m tiles; memset once zeroes pads + halo.
            m_tiles = [
                mpool.tile([P, msz], F32, tag=f"m{bi}", name=f"m{bi}")
                for bi in range(bpc)
            ]
            for mt in m_tiles:
                nc.gpsimd.memset(mt[:], 0.0)

            for bi in range(bpc):
                mt = m_tiles[bi]
                mt3 = mt[:].rearrange("p (r w) -> p r w", w=rowp)  # [P,mrows,rowp]

                # ---- stage 1: channel max into m tile ----
                # One DVE pool_max per chunk (fewest ops; reduce is 1 elem/
                # cycle/lane regardless of decomposition), writing straight
                # into the padded m layout.
                for r in range(ph):
                    for q in range(nq):
                        xt = xpool.tile([P, qw * c], F32, tag="xt", name="xt")
                        nc.sync.dma_start(
                            out=xt[:],
                            in_=xr[bi, :, r, q * qw * c:(q + 1) * qw * c],
                        )
                        x3 = xt[:].rearrange("p (w c) -> p w c", c=c)
                        nc.vector.reduce_max(
                            mt3[:, SPAN + r, SPAN + q * qw: SPAN + (q + 1) * qw],
                            x3,
                            axis=mybir.AxisListType.X,
                        )

                # ---- stage 2: halo rows via partition-shifted SBUF DMA ----
                # partition p rows [0:2) <- partition p-1 rows [ph:ph+2)
                nc.scalar.dma_start(
                    out=mt[1:P, 0:SPAN * rowp],
                    in_=mt[0:P - 1, ph * rowp:(ph + SPAN) * rowp],
                )

                # ---- stage 3: 7-shift max ----
                acc = opool.tile([P, ph * w], F32, tag="acc", name="acc")
                a3 = acc[:].rearrange("p (r w) -> p r w", w=w)

                def opnd(dh, dw):
                    return mt3[:, SPAN - dh:SPAN - dh + ph, SPAN - dw:SPAN - dw + w]

                shifts = [(0, 0), (1, 0), (0, 1), (1, 1), (2, 0), (0, 2), (2, 2)]
                nc.vector.tensor_max(a3, opnd(*shifts[0]), opnd(*shifts[1]))
                for s in shifts[2:]:
                    nc.vector.tensor_max(a3, a3, opnd(*s))

                nc.scalar.dma_start(out=outr[bi], in_=acc[:])

    # run_bass_via_pjrt binds the bass_exec primitive without finalizing;
    # Bacc needs finalize() -> compile() for register allocation and the
    # TRN2 one-wait-per-instruction semaphore legalization.
    nc.finalize()
    return nc


_NC_CACHE = {}


def _get_nc():
    key = "full"
    if key not in _NC_CACHE:
        _NC_CACHE[key] = build_nc(B // NCORES, H, W, C, ph=4, qw=256)
    return _NC_CACHE[key]


def _run(x, trace=False):
    """Run the SPMD kernel on 8 cores. Returns (out, BassKernelResults)."""
    from concourse.bass_utils import run_bass_kernel_spmd

    x = np.ascontiguousarray(np.asarray(x), dtype=np.float32)
    assert x.shape == (B, H, W, C)
    bpc = B // NCORES
    nc = _get_nc()
    in_maps = [
        {"x": np.ascontiguousarray(x[i * bpc:(i + 1) * bpc])} for i in range(NCORES)
    ]
    res = run_bass_kernel_spmd(nc, in_maps, list(range(NCORES)), trace=trace)
    out = np.concatenate([res.results[i]["out"] for i in range(NCORES)], axis=0)
    return out, res


def kernel(x):
    out, _ = _run(x, trace=False)
    return out
